# revision 18
# baseline (speedup 1.0000x reference)
"""AnomalyAttention Trainium2 kernel — 8-way head-parallel SPMD.

Each of the 8 NeuronCores computes one attention head end-to-end:
  QT/KT/VT projections, scoresT stream with fused normalize+exp (ScalarE),
  softmax denominators via ones-matmul (PE), Z accumulation (PE),
  prior P via an exact integer rank-3 matmul for (n-m)^2 + one Exp pass,
  and a row-split W0 partial output projection.

Host side: shard by head, run SPMD on cores 0-7, then unshard
(stack S/P per head, sum the row-parallel W0 partials).
"""

import sys

for _p in ("/opt/trn_rl_repo",):
    if _p not in sys.path:
        sys.path.insert(0, _p)

import numpy as np

import concourse.bass as bass
from concourse import bacc
import concourse.mybir as mybir
import concourse.tile as tile
from concourse.bass_utils import run_bass_kernel_spmd
from concourse.masks import make_identity

N, D, H, DK = 2048, 1024, 8, 128
NDC = D // 128      # 8 d-chunks
NCH = N // 128      # 16 token chunks of 128
NB = N // 512       # 4 token blocks of 512
FP32 = mybir.dt.float32
AF = mybir.ActivationFunctionType

_COMPILED = {}


def build_bass():
    import os
    phases = os.environ.get("K_PHASES", "PS")
    nc = bacc.Bacc()

    # ---- I/O ----------------------------------------------------------------
    xt = nc.dram_tensor("xt", [128, NDC, N], FP32, kind="ExternalInput")
    wq = nc.dram_tensor("wq", [128, NDC, DK], FP32, kind="ExternalInput")
    wk = nc.dram_tensor("wk", [128, NDC, DK], FP32, kind="ExternalInput")
    wv = nc.dram_tensor("wv", [128, NDC, DK], FP32, kind="ExternalInput")
    ws = nc.dram_tensor("ws", [128, NDC], FP32, kind="ExternalInput")
    w0 = nc.dram_tensor("w0", [DK, D], FP32, kind="ExternalInput")
    b0 = nc.dram_tensor("b0", [128, NDC], FP32, kind="ExternalInput")
    ntr = nc.dram_tensor("ntr", [3, N], FP32, kind="ExternalInput")   # n^2, -2n, 1
    rtr = nc.dram_tensor("rtr", [3, N], FP32, kind="ExternalInput")   # 1, m, m^2

    s_out = nc.dram_tensor("s_out", [NB, N, 512], FP32, kind="ExternalOutput")
    p_out = nc.dram_tensor("p_out", [N, N], FP32, kind="ExternalOutput")
    o_out = nc.dram_tensor("o_out", [D, N], FP32, kind="ExternalOutput")

    from contextlib import ExitStack

    with tile.TileContext(nc, linearize=bool(os.environ.get("K_LIN"))) as tc, ExitStack() as es:
        consts = es.enter_context(tc.tile_pool(name="consts", bufs=1))
        statsp = es.enter_context(tc.tile_pool(name="stats", bufs=1))
        projp = es.enter_context(tc.tile_pool(name="proj", bufs=1))

        ident = consts.tile([128, 128], FP32)
        make_identity(nc, ident)
        ones_m = consts.tile([128, 1], FP32)
        nc.vector.memset(ones_m, 1.0)
        ones_k1 = consts.tile([1, 128], FP32)
        nc.vector.memset(ones_k1, 1.0)
        w0_sb = consts.tile([DK, D], FP32)
        nc.sync.dma_start(out=w0_sb, in_=w0[:])
        b0_sb = consts.tile([128, NDC], FP32)
        nc.sync.dma_start(out=b0_sb, in_=b0[:])
        nt_sb = consts.tile([3, N], FP32)
        nc.sync.dma_start(out=nt_sb, in_=ntr[:])
        rt_sb = consts.tile([3, N], FP32)
        nc.sync.dma_start(out=rt_sb, in_=rtr[:])
        ws_sb = consts.tile([128, NDC], FP32)
        nc.sync.dma_start(out=ws_sb, in_=ws[:])

        # columnar per-token vectors [128, NCH] (token t = c*128 + p)
        s1c = statsp.tile([128, NCH], FP32)
        s2c = statsp.tile([128, NCH], FP32)
        muc = statsp.tile([128, NCH], FP32)
        sdinvc = statsp.tile([128, NCH], FP32)
        biasc = statsp.tile([128, NCH], FP32)
        sgc = statsp.tile([128, NCH], FP32)
        isgc = statsp.tile([128, NCH], FP32)
        pscalec = statsp.tile([128, NCH], FP32)
        ginvc = statsp.tile([128, NCH], FP32)
        growc = statsp.tile([128, NCH], FP32)
        ksum = statsp.tile([128, 1], FP32)
        tmpc = statsp.tile([128, NCH], FP32)
        scr = statsp.tile([128, 128], FP32)

        qt_sb = projp.tile([128, N], FP32)
        kt_sb = projp.tile([128, N], FP32)
        v_sb = projp.tile([128, N], FP32)   # token-major V

        # ==== Phase A: load x/W, projections, sigma, transposes, stats ======
        with (
            tc.tile_pool(name="xw", bufs=1) as xw,
            tc.tile_pool(name="psA", bufs=1, space="PSUM") as psA,
            tc.tile_pool(name="psG", bufs=1, space="PSUM") as psG,
            tc.tile_pool(name="psT", bufs=1, space="PSUM") as psT,
            tc.tile_pool(name="psSg", bufs=2, space="PSUM") as psSg,
        ):
            q_sb = xw.tile([128, N], FP32)   # token-major Q  [m, k]
            k_sb = xw.tile([128, N], FP32)   # token-major K
            g_sb = xw.tile([128, 129], FP32)
            xt_sb = xw.tile([128, NDC, N], FP32)
            nc.sync.dma_start(out=xt_sb, in_=xt[:])
            wq_sb = xw.tile([128, NDC, DK], FP32)
            nc.sync.dma_start(out=wq_sb, in_=wq[:])
            wk_sb = xw.tile([128, NDC, DK], FP32)
            nc.sync.dma_start(out=wk_sb, in_=wk[:])
            wv_sb = xw.tile([128, NDC, DK], FP32)
            nc.sync.dma_start(out=wv_sb, in_=wv[:])

            # projections: d-outer, n-block inner; PSUM [128, 4, 512] = 4 banks
            for w_sb, t_sb in ((wq_sb, qt_sb), (wk_sb, kt_sb), (wv_sb, v_sb)):
                pp = psA.tile([128, NB, 512], FP32, tag="proj")
                for dc in range(NDC):
                    for b in range(NB):
                        nc.tensor.matmul(
                            pp[:, b],
                            w_sb[:, dc],
                            xt_sb[:, dc, b * 512:(b + 1) * 512],
                            start=(dc == 0),
                            stop=(dc == NDC - 1),
                        )
                if t_sb is v_sb:
                    # VT only needed transiently; transpose to token-major V
                    vt_tmp = xw.tile([128, N], FP32)
                    for b in range(NB):
                        nc.scalar.copy(vt_tmp[:, b * 512:(b + 1) * 512], pp[:, b])
                    for c in range(NCH):
                        tp = psT.tile([128, 128], FP32, tag="tp")
                        nc.tensor.transpose(
                            tp, vt_tmp[:, c * 128:(c + 1) * 128], ident
                        )
                        nc.vector.tensor_copy(v_sb[:, c * 128:(c + 1) * 128], tp)
                else:
                    for b in range(NB):
                        nc.scalar.copy(t_sb[:, b * 512:(b + 1) * 512], pp[:, b])

            # sigma: row [1, N] via M=1 matmuls, then columnar via K=1
            # broadcast + identity-mask diagonal extraction
            sub = int(os.environ.get("K_SUB", "3"))
            srow = xw.tile([1, N], FP32)
            for nb in range(NB if sub >= 2 else 0):
                sp = psSg.tile([1, 512], FP32, tag="sg")
                for dc in range(NDC):
                    nc.tensor.matmul(
                        sp,
                        ws_sb[:, dc:dc + 1],
                        xt_sb[:, dc, nb * 512:(nb + 1) * 512],
                        start=(dc == 0),
                        stop=(dc == NDC - 1),
                    )
                nc.scalar.copy(srow[:, nb * 512:(nb + 1) * 512], sp)
            for c in range(NCH if sub >= 2 else 0):
                bc = psT.tile([128, 128], FP32, tag="tp")
                nc.tensor.matmul(
                    bc, ones_k1, srow[:, c * 128:(c + 1) * 128],
                    start=True, stop=True,
                )
                nc.vector.tensor_mul(scr, bc, ident)
                nc.vector.tensor_reduce(
                    sgc[:, c:c + 1], scr,
                    axis=mybir.AxisListType.X, op=mybir.AluOpType.add,
                )

            # token-major Q, K via PE transpose
            for src, dst in (((qt_sb, q_sb), (kt_sb, k_sb)) if sub >= 2 else ()):
                for c in range(NCH):
                    tp = psT.tile([128, 128], FP32, tag="tp")
                    nc.tensor.transpose(tp, src[:, c * 128:(c + 1) * 128], ident)
                    nc.vector.tensor_copy(dst[:, c * 128:(c + 1) * 128], tp)

            # ---- scores row stats without touching N^2 -----------------------
            if sub < 3:
                nc.vector.memset(s1c, 1.0); nc.vector.memset(s2c, 3.0)
                nc.vector.memset(sgc, 1.0)
            # ksum[k] = sum_m K[m, k]
            if sub >= 3:
             nc.vector.tensor_reduce(
                ksum, kt_sb, axis=mybir.AxisListType.X, op=mybir.AluOpType.add
            )
            # G = K^T K  [k, k']
            if sub >= 3:
             gp = psG.tile([128, 128], FP32, tag="gram")
             for c in range(NCH):
                nc.tensor.matmul(
                    gp,
                    k_sb[:, c * 128:(c + 1) * 128],
                    k_sb[:, c * 128:(c + 1) * 128],
                    start=(c == 0),
                    stop=(c == NCH - 1),
                )
             nc.vector.tensor_copy(g_sb[:, 0:128], gp)
             nc.vector.tensor_copy(g_sb[:, 128:129], ksum)
             for c in range(NCH):
                cs = slice(c * 128, (c + 1) * 128)
                tp = psG.tile([128, 129], FP32, tag="gram")
                nc.tensor.matmul(tp, qt_sb[:, cs], g_sb, start=True, stop=True)
                nc.scalar.copy(s1c[:, c:c + 1], tp[:, 128:129])
                # S2[t] = sum_k (Q G)[t, k] * Q[t, k]
                nc.vector.tensor_mul(scr, tp[:, 0:128], q_sb[:, cs])
                nc.vector.tensor_reduce(
                    s2c[:, c:c + 1], scr,
                    axis=mybir.AxisListType.X, op=mybir.AluOpType.add,
                )

        # stats vector math (columnar, cheap)
        nc.vector.tensor_scalar_mul(muc, s1c, 1.0 / N)
        nc.vector.tensor_mul(tmpc, muc, s1c)
        nc.vector.tensor_sub(s2c, s2c, tmpc)
        nc.vector.tensor_scalar_mul(s2c, s2c, 1.0 / (N - 1))  # unbiased var
        nc.scalar.sqrt(tmpc, s2c)
        nc.vector.reciprocal(sdinvc, tmpc)                    # 1/sd
        nc.vector.tensor_mul(biasc, muc, sdinvc)
        nc.vector.tensor_scalar_mul(biasc, biasc, -1.0)       # -mu/sd

        # sigma -> -0.5 / clamp(sigma, 1e-3)^2
        nc.vector.tensor_scalar_max(sgc, sgc, 0.001)
        nc.vector.tensor_mul(isgc, sgc, sgc)
        nc.vector.reciprocal(isgc, isgc)
        nc.vector.tensor_scalar_mul(pscalec, isgc, -0.5)

        # created after the xw pool is released so it can reuse that space
        ebigp = es.enter_context(tc.tile_pool(name="ebig", bufs=1))
        e_big = ebigp.tile([128, NCH, N], FP32)  # E^T tiles, 16 x [128, 2048]

        # ==== Phase B: prior P (natural orientation, n on partitions) ========
        if "P" in phases:
         with (
            tc.tile_pool(name="psP", bufs=2, space="PSUM") as psP,
            tc.tile_pool(name="gP", bufs=2) as gP,
        ):
            for c in range(NCH):
                cs = slice(c * 128, (c + 1) * 128)
                ap = psP.tile([128, NB, 512], FP32, tag="parg")
                for mb in range(NB):
                    nc.tensor.matmul(
                        ap[:, mb],
                        nt_sb[:, cs],
                        rt_sb[:, mb * 512:(mb + 1) * 512],
                        start=True,
                        stop=True,
                    )
                gt = gP.tile([128, N], FP32, tag="gt")
                nc.scalar.activation(
                    gt,
                    ap.rearrange("p b w -> p (b w)"),
                    AF.Exp,
                    bias=0.0,
                    scale=pscalec[:, c:c + 1],
                    accum_out=growc[:, c:c + 1],
                )
                nc.vector.reciprocal(ginvc[:, c:c + 1], growc[:, c:c + 1])
                nc.vector.tensor_scalar_mul(gt, gt, ginvc[:, c:c + 1])
                nc.sync.dma_start(out=p_out[cs, :], in_=gt)

        # ==== Phase C: attention stream (T-orientation) ======================
        if "S" in phases:
         with (
            tc.tile_pool(name="psS", bufs=2, space="PSUM") as psS,
            tc.tile_pool(name="psZ", bufs=1, space="PSUM") as psZ,
            tc.tile_pool(name="psD", bufs=1, space="PSUM") as psD,
            tc.tile_pool(name="psO", bufs=2, space="PSUM") as psO,
            tc.tile_pool(name="sm", bufs=2) as sm,
            tc.tile_pool(name="smv", bufs=1) as smv,
            tc.tile_pool(name="ob", bufs=1) as ob,
        ):
            for b in range(NB):
                bs = slice(b * 512, (b + 1) * 512)
                zun = psZ.tile([128, 512], FP32, tag="zun")
                dns = psD.tile([1, 512], FP32, tag="dns")
                for c in range(NCH):
                    cs = slice(c * 128, (c + 1) * 128)
                    scs = psS.tile([128, 512], FP32, tag="scs")
                    nc.tensor.matmul(
                        scs, kt_sb[:, cs], qt_sb[:, bs], start=True, stop=True
                    )
                    # E^T = exp((scores - mu[m]) / sd[m]); m is the partition
                    nc.scalar.activation(
                        e_big[:, c, bs],
                        scs,
                        AF.Exp,
                        bias=biasc[:, c:c + 1],
                        scale=sdinvc[:, c:c + 1],
                    )
                    nc.tensor.matmul(
                        zun,
                        v_sb[:, cs],
                        e_big[:, c, bs],
                        start=(c == 0),
                        stop=(c == NCH - 1),
                    )
                    nc.tensor.matmul(
                        dns,
                        ones_m,
                        e_big[:, c, bs],
                        start=(c == 0),
                        stop=(c == NCH - 1),
                    )
                # denominators -> 1/denom broadcast to all partitions
                dsb = smv.tile([1, 512], FP32, tag="dsb")
                nc.scalar.copy(dsb, dns)
                dinv = smv.tile([1, 512], FP32, tag="dinv")
                nc.vector.reciprocal(dinv, dsb)
                dbp = psD.tile([128, 512], FP32, tag="dbp")
                nc.tensor.matmul(dbp, ones_k1, dinv, start=True, stop=True)
                dinvb = sm.tile([128, 512], FP32, tag="dinvb")
                nc.vector.tensor_copy(dinvb, dbp)

                # ZT = Zun * dinv[n], then partial out^T = W0h^T @ Z^T (+ b0)
                ztsb = sm.tile([128, 512], FP32, tag="ztsb")
                nc.vector.tensor_mul(ztsb, zun, dinvb)
                obig = ob.tile([128, NDC, 512], FP32, tag="obig")
                for dc in range(NDC):
                    op_ = psO.tile([128, 512], FP32, tag="op")
                    nc.tensor.matmul(
                        op_,
                        w0_sb[:, dc * 128:(dc + 1) * 128],
                        ztsb,
                        start=True,
                        stop=True,
                    )
                    nc.vector.tensor_scalar_add(obig[:, dc], op_, b0_sb[:, dc:dc + 1])
                nc.sync.dma_start(
                    out=o_out[:].rearrange("(dc p) n -> p dc n", p=128)[:, :, bs],
                    in_=obig,
                )

                # S = E * dinv[n] (in place), one 4 MB DMA per block
                for c in range(NCH):
                    nc.vector.tensor_mul(
                        e_big[:, c, bs], e_big[:, c, bs], dinvb
                    )
                nc.sync.dma_start(
                    out=s_out[b].rearrange("(c p) w -> p c w", p=128),
                    in_=e_big[:, :, bs],
                )

    nc.compile()
    return nc


def _host_inputs(x, Wq, Wk, Wv, Ws, W0, b0):
    x = np.asarray(x, np.float32)
    sc = np.float32(1.0 / np.sqrt(np.float32(DK)))
    xt_r = np.ascontiguousarray(x.T.reshape(NDC, 128, N).transpose(1, 0, 2))
    n_i = np.arange(N, dtype=np.float32)
    ntr = np.stack([n_i * n_i, -2.0 * n_i, np.ones_like(n_i)]).astype(np.float32)
    rtr = np.stack([np.ones_like(n_i), n_i, n_i * n_i]).astype(np.float32)

    def _w(w, scale=False):
        w = np.asarray(w, np.float32)
        if scale:
            w = w * sc
        return np.ascontiguousarray(w.reshape(NDC, 128, DK).transpose(1, 0, 2))

    in_maps = []
    for h in range(H):
        in_maps.append({
            "xt": xt_r,
            "wq": _w(Wq[h], scale=True),
            "wk": _w(Wk[h]),
            "wv": _w(Wv[h]),
            "ws": np.ascontiguousarray(
                np.asarray(Ws[h], np.float32).reshape(NDC, 128).T),
            "w0": np.ascontiguousarray(np.asarray(W0[h * DK:(h + 1) * DK], np.float32)),
            "b0": (np.ascontiguousarray(np.asarray(b0, np.float32).reshape(NDC, 128).T)
                   if h == 0 else np.zeros((128, NDC), np.float32)),
            "ntr": ntr,
            "rtr": rtr,
        })
    return in_maps


def kernel(x, Wq, Wk, Wv, Ws, W0, b0, trace=False):
    if "nc" not in _COMPILED:
        _COMPILED["nc"] = build_bass()
    nc = _COMPILED["nc"]
    in_maps = _host_inputs(x, Wq, Wk, Wv, Ws, W0, b0)
    kw = {}
    if trace:
        import pathlib
        pathlib.Path("/tmp/ntff_prof").mkdir(exist_ok=True)
        kw = dict(tmpdir="/tmp/ntff_prof")
    res = run_bass_kernel_spmd(nc, in_maps, list(range(H)), trace=trace, **kw)
    outs = res.results

    o_sum = np.zeros((D, N), np.float32)
    S = np.empty((H, N, N), np.float32)
    P = np.empty((H, N, N), np.float32)
    for h in range(H):
        o_sum += outs[h]["o_out"]
        s_raw = outs[h]["s_out"]                      # [NB, m, 512] (n-blocked, T)
        S[h] = s_raw.transpose(1, 0, 2).reshape(N, N).T
        P[h] = outs[h]["p_out"]
    out = np.ascontiguousarray(o_sum.T)
    if trace:
        kernel.last_exec_ns = res.exec_time_ns
    return out, P, S
